# revision 30
# baseline (speedup 1.0000x reference)
"""Trainium2 Bass kernel for nn_BoundaryLoss (8-core data-parallel).

Math (see reference): loss = (1/C) * sum_c mean_{b,h,w} |pred_sdf_c - tgt_sdf_c|.

For randn pred (logit spread < 15) softmax probs are never exactly 0/1, so
pred_sdf == 0 and only the target side matters (host verifies, else exact
fallback).  Per pixel p with own class g = t(p):

  contribution = sum_c d_c(p) + min_{c != g} d_c(p)

with d_c = min(5, dist to {target==c}) and d_g = 0.

Device algorithm (exp-space capped EDT, all distances as D^2 in [0,25]):
  X_c = Wx (*) Wy (*) m_c  with banded Gaussian weights e^{20-5k^2}, |k|<=3
  (pass-1 = y-conv fused with transpose on PE; drain to bf16; pass-2 = x-conv
  on PE).  A third diagonal matmul subtracts K*m_c (K = e^42) so own-class
  pixels go NEGATIVE: their f32 sign bit makes the extracted value land in a
  constant cluster (round -> 35) that the host subtracts exactly, and they
  self-exclude from the d_minus min (as +35 > any real 25-capped value).

  Extraction needs only the top 16 bits of each f32: for X = e^{40-5v+xi},
  v = round(A*hi16 + B) EXACTLY (margin ~0.27; hi16 read as strided int16
  view of PSUM).  One DVE tensor_scalar does the affine + RNE cast; ACT
  Sqrt(v)+accum produces sum sqrt(v) partials; a 2-level tensor_tensor min
  tree gives v_neq = min_{c != g} v_c, then Sqrt+accum.

  Band +-3 drops D^2 > 18 configurations at frequency ~(3/4)^48 per pixel
  (expected <5 pixels across the whole batch, each off by <1.1 in d): error
  ~1e-6 on the loss.  Validated end-to-end in numpy at rel err 8.4e-5.

Per core: 2 images.  Output: [128, 16] f32 per-partition partial sums.
"""

import ml_dtypes
import numpy as np

import concourse.bacc as bacc
import concourse.bass as bass
import concourse.mybir as mybir
from concourse.mybir import AluOpType as Op
from concourse.tile import TileContext

P = 128
H = W = 512
YB = H // P          # 4 blocks per image dimension
C = 4                # classes
BPC = 2              # images per core
NCORES = 8
B_TOTAL = BPC * NCORES
RAD = 3              # conv band radius

BETA = 5.0
BW = 20.0            # per-pass exp bias (two passes -> e^40 at distance 0)
K_EXCL = 42.0        # own-class exclusion: subtract e^42 * m

A_COEF = -float(np.log(2.0)) / 640.0
B_COEF = (40.0 + 127.0 * float(np.log(2.0))) / 5.0 + 0.2277
GAMMA_HAT = float(np.sqrt(35.0))   # constant own-class cluster, verified

F32 = mybir.dt.float32
BF16 = mybir.dt.bfloat16
I32 = mybir.dt.int32
I16 = mybir.dt.int16
Act = mybir.ActivationFunctionType

# weight-matrix column layout in the packed wmats input
WCOMB_OFF, DIAG_OFF = 0, 134
WM_COLS = 134 + 128


def _band_val(d):
    if abs(d) > RAD:
        return 0.0
    return float(np.exp(BW - BETA * d * d))


def _host_wmats():
    """[128, 262] bf16: W_COMB [128,134] | DIAG [128,128] (-e^42).

    W_COMB[p, q] = band(q - 3 - p): matmul with stationary = data block j
    and moving = W_COMB writes output window [128j-3, 128j+131) — the
    3-col halo into each neighbor block plus the 128-col body, all from
    one quadrant-aligned full-height stationary.
    """
    wm = np.zeros((P, WM_COLS), np.float32)
    for p_ in range(P):
        for q in range(134):
            wm[p_, WCOMB_OFF + q] = _band_val(q - 3 - p_)
    for p_ in range(P):
        wm[p_, DIAG_OFF + p_] = -float(np.exp(K_EXCL))
    return wm.astype(ml_dtypes.bfloat16)


def _build_nc():
    nc = bacc.Bacc("TRN2", target_bir_lowering=False, debug=False)
    tgt_d = nc.dram_tensor("target", [BPC, H, W], I32, kind="ExternalInput")
    wm_d = nc.dram_tensor("wmats", [P, WM_COLS], BF16, kind="ExternalInput")
    osum_d = nc.dram_tensor("osum", [P, BPC * YB], F32, kind="ExternalOutput")

    with TileContext(nc) as tc:
        with (
            tc.tile_pool(name="const", bufs=1) as cpool,
            tc.tile_pool(name="tgt", bufs=4) as tgt_pool,
            tc.tile_pool(name="tgtb", bufs=8) as tb_pool,
            tc.tile_pool(name="mask", bufs=2 * YB) as m_pool,
            tc.tile_pool(name="st", bufs=2 * YB) as st_pool,
            tc.tile_pool(name="v16", bufs=3) as v_pool,
            tc.tile_pool(name="mins", bufs=3) as mn_pool,
            tc.tile_pool(name="scratch", bufs=2) as s_pool,
            tc.tile_pool(name="acc", bufs=1) as a_pool,
            tc.tile_pool(name="ps", bufs=2, space="PSUM") as ps_pool,
        ):
            wtile = cpool.tile([P, WM_COLS], BF16)
            nc.sync.dma_start(wtile, wm_d[:, :])
            wc = wtile[:, WCOMB_OFF:WCOMB_OFF + 134]
            w_diag = wtile[:, DIAG_OFF:DIAG_OFF + P]
            bias_b = cpool.tile([P, 1], F32)
            nc.vector.memset(bias_b, B_COEF)

            def banded_conv(ps_row, stat_of, fresh):
                """Banded conv into psum row `ps_row` ([128, W] f32);
                stat_of(j) gives the [128,128] stationary for block j.

                fresh=True: regions initialized by start=True pieces (7
                matmuls, 6-col overlap pieces accumulate).  fresh=False:
                the bank was already initialized (diag); 4 accumulating
                window matmuls, one per block."""
                for j in range(YB):
                    lo = j * P - 3
                    stat = stat_of(j)
                    last = j == YB - 1
                    if not fresh:
                        if j == 0:
                            nc.tensor.matmul(ps_row[:, 0:131], stat,
                                             wc[:, 3:134],
                                             start=False, stop=False,
                                             skip_group_check=True)
                        else:
                            hi_end = min(lo + 134, W)
                            nc.tensor.matmul(ps_row[:, lo:hi_end], stat,
                                             wc[:, 0:hi_end - lo],
                                             start=False, stop=last,
                                             skip_group_check=True)
                        continue
                    if j == 0:
                        nc.tensor.matmul(ps_row[:, 0:131], stat, wc[:, 3:134],
                                         start=True, stop=False,
                                         skip_group_check=True)
                        continue
                    hi_end = min(lo + 134, W)
                    nc.tensor.matmul(ps_row[:, lo:lo + 6], stat, wc[:, 0:6],
                                     start=False, stop=False,
                                     skip_group_check=True)
                    nc.tensor.matmul(ps_row[:, lo + 6:hi_end], stat,
                                     wc[:, 6:6 + (hi_end - lo - 6)],
                                     start=True, stop=last,
                                     skip_group_check=True)

            osum = a_pool.tile([P, BPC * YB], F32)

            def stage_load(b):
                """DMA targets + i32->bf16 casts (image 0 on the idle
                Scalar engine to shorten the pipeline head)."""
                tgtb = []
                for yb in range(YB):
                    tgt = tgt_pool.tile([P, W], I32)
                    nc.sync.dma_start(tgt, tgt_d[b, yb * P:(yb + 1) * P, :])
                    tb = tb_pool.tile([P, W], BF16)
                    if b == 0:
                        nc.scalar.activation(tb, tgt, Act.Copy)
                    else:
                        nc.gpsimd.tensor_copy(tb, tgt)
                    tgtb.append(tb)
                return tgtb

            def stage_masks(tgtb):
                mw = []
                for yb in range(YB):
                    mwt = m_pool.tile([P, C, W], BF16)
                    for c in range(C):
                        nc.vector.tensor_scalar(mwt[:, c], tgtb[yb], float(c),
                                                None, Op.is_equal)
                    mw.append(mwt)
                return mw

            def stage_pass1(mw):
                """y-conv fused with transpose; drain to bf16 (V/S split)."""
                st = []
                for xb in range(YB):
                    syt = ps_pool.tile([P, C, W], F32, tag="ps")
                    for c in range(C):
                        banded_conv(syt[:, c],
                                    lambda j: mw[j][:, c, xb * P:(xb + 1) * P],
                                    fresh=True)
                    stx = st_pool.tile([P, C, W], BF16)
                    if xb % 2 == 0:
                        nc.scalar.activation(stx, syt, Act.Copy)
                    else:
                        nc.vector.tensor_copy(stx, syt)
                    st.append(stx)
                return st

            def stage_pass2(b, mw, st):
                """x-conv + diag exclusion; extract; min tree; fused sqrt."""
                for ys in range(YB):
                    xp = ps_pool.tile([P, C, W], F32, tag="ps")
                    for c in range(C):
                        nc.tensor.matmul(
                            xp[:, c, :], w_diag, mw[ys][:, c, :],
                            start=True, stop=False, skip_group_check=True,
                        )
                        banded_conv(xp[:, c],
                                    lambda xb: st[xb][:, c, ys * P:(ys + 1) * P],
                                    fresh=False)

                    # v = round(A*hi16 + B) from the PSUM top halves; planes
                    # 0..3 = classes, plane 4 = min tree for d_neq.
                    xpap = xp[:].bitcast(I16)
                    v16 = v_pool.tile([P, C + 1, W], I16)
                    last = b == BPC - 1 and ys == YB - 1
                    if last:
                        # final slice: split V/S so the pipeline tail drains
                        # in parallel
                        hi_a = bass.AP(xpap.tensor, xpap.offset + 1,
                                       [xpap.ap[0], [2 * W, 2], [2, W]])
                        hi_b = bass.AP(xpap.tensor, xpap.offset + 1 + 4 * W,
                                       [xpap.ap[0], [2 * W, 2], [2, W]])
                        nc.vector.tensor_scalar(v16[:, 0:2], hi_a, A_COEF,
                                                B_COEF, Op.mult, Op.add)
                        nc.scalar.activation(v16[:, 2:4], hi_b, Act.Identity,
                                             bias=bias_b[:], scale=A_COEF)
                    else:
                        hi = bass.AP(xpap.tensor, xpap.offset + 1,
                                     [xpap.ap[0], [2 * W, C], [2, W]])
                        if ys == YB - 1:
                            nc.scalar.activation(v16[:, 0:C], hi, Act.Identity,
                                                 bias=bias_b[:], scale=A_COEF)
                        else:
                            nc.vector.tensor_scalar(v16[:, 0:C], hi, A_COEF,
                                                    B_COEF, Op.mult, Op.add)

                    pr = mn_pool.tile([P, 2, W], I16)
                    nc.vector.tensor_tensor(pr[:], v16[:, 0:2], v16[:, 2:4],
                                            Op.min)
                    nc.vector.tensor_tensor(v16[:, C], pr[:, 0], pr[:, 1],
                                            Op.min)

                    col = b * YB + ys
                    dscr = s_pool.tile([P, C + 1, W], BF16)
                    nc.scalar.activation(dscr[:], v16[:], Act.Sqrt,
                                         accum_out=osum[:, col:col + 1])

            # two-image software pipeline: both pass-1s run back to back so
            # the Scalar drains never queue behind pass-2 sqrt work, and the
            # Vector mask work for image 1 hides under image 0's conv.
            tgtb0 = stage_load(0)
            tgtb1 = stage_load(1)
            mw0 = stage_masks(tgtb0)
            st0 = stage_pass1(mw0)
            mw1 = stage_masks(tgtb1)
            st1 = stage_pass1(mw1)
            stage_pass2(0, mw0, st0)
            stage_pass2(1, mw1, st1)

            nc.sync.dma_start(osum_d[:, :], osum)

    nc.compile()
    return nc


_NC = None
_WM = None


def _get_nc():
    global _NC
    if _NC is None:
        _NC = _build_nc()
    return _NC


def _get_wm():
    global _WM
    if _WM is None:
        _WM = _host_wmats()
    return _WM


def _exact_fallback(pred, target):
    """Exact numpy implementation of the reference (adversarial inputs only)."""
    THETA0, THETA, R = 3.0, 5.0, 5
    offs = [(dy, dx, float(np.hypot(dy, dx)))
            for dy in range(-R, R + 1) for dx in range(-R, R + 1)
            if np.hypot(dy, dx) <= THETA]

    def capped_edt(ts):
        B, Hh, Ww = ts.shape
        pad = np.zeros((B, Hh + 2 * R, Ww + 2 * R), bool)
        pad[:, R:-R, R:-R] = ts
        d = np.full((B, Hh, Ww), THETA, np.float32)
        for dy, dx, dist in offs:
            win = pad[:, R + dy:R + dy + Hh, R + dx:R + dx + Ww]
            d = np.minimum(d, np.where(win, np.float32(dist), np.float32(THETA)))
        return d

    def compute_sdf(mask):
        sdf_pos = capped_edt(mask == 1.0)
        sdf_neg = capped_edt(mask == 0.0)
        sdf = np.clip(sdf_pos - sdf_neg, -THETA, THETA) / THETA
        empty = mask.sum(axis=(1, 2)) == 0.0
        return np.where(empty[:, None, None], np.float32(THETA0), sdf).astype(np.float32)

    x = pred.astype(np.float32)
    x = x - x.max(axis=1, keepdims=True)
    ex = np.exp(x)
    p = ex / ex.sum(axis=1, keepdims=True)
    Cn = pred.shape[1]
    loss = np.float32(0.0)
    for c in range(Cn):
        ps = compute_sdf(p[:, c].astype(np.float32))
        ts = compute_sdf((target == c).astype(np.float32))
        loss += np.abs(ps - ts).mean(dtype=np.float32)
    return np.float32(loss / Cn)


def kernel(pred: np.ndarray, target: np.ndarray) -> np.ndarray:
    pred = np.asarray(pred)
    target = np.asarray(target)

    gap_ok = float(pred.max()) - float(pred.min()) < 15.0
    tgt_ok = bool(((target >= 0) & (target < C)).all())
    present = np.array([[(target[b] == c).any() for c in range(C)]
                        for b in range(B_TOTAL)])
    if not (gap_ok and tgt_ok and present.all()):
        return _exact_fallback(pred, target)

    from concourse.bass_utils import run_bass_kernel_spmd

    nc = _get_nc()
    wm = _get_wm()
    in_maps = [
        {"target": np.ascontiguousarray(target[i * BPC:(i + 1) * BPC]),
         "wmats": wm}
        for i in range(NCORES)
    ]
    try:
        res = run_bass_kernel_spmd(nc, in_maps, list(range(NCORES))).results
    except Exception:
        import time as _time
        _time.sleep(3.0)
        res = run_bass_kernel_spmd(nc, in_maps, list(range(NCORES))).results

    npx = H * W
    total = 0.0
    for core in range(NCORES):
        total += float(res[core]["osum"].astype(np.float64).sum())
    total -= B_TOTAL * npx * GAMMA_HAT
    loss = total / (5.0 * npx * B_TOTAL * C)
    return np.float32(loss)


# revision 32
# speedup vs baseline: 1.0954x; 1.0954x over previous
"""Trainium2 Bass kernel for nn_BoundaryLoss (8-core data-parallel).

Math (see reference): loss = (1/C) * sum_c mean_{b,h,w} |pred_sdf_c - tgt_sdf_c|.

For randn pred (logit spread < 15) softmax probs are never exactly 0/1, so
pred_sdf == 0 and only the target side matters (host verifies, else exact
fallback).  Per pixel p with own class g = t(p):

  contribution = sum_c d_c(p) + min_{c != g} d_c(p)

with d_c = min(5, dist to {target==c}) and d_g = 0.

Device algorithm (exp-space capped EDT, all distances as D^2 in [0,25]):
  X_c = Wx (*) Wy (*) m_c  with banded Gaussian weights e^{20-5k^2}, |k|<=3
  (pass-1 = y-conv fused with transpose on PE; drain to bf16; pass-2 = x-conv
  on PE).  A third diagonal matmul subtracts K*m_c (K = e^42) so own-class
  pixels go NEGATIVE: their f32 sign bit makes the extracted value land in a
  constant cluster (round -> 35) that the host subtracts exactly, and they
  self-exclude from the d_minus min (as +35 > any real 25-capped value).

  Extraction needs only the top 16 bits of each f32: for X = e^{40-5v+xi},
  v = round(A*hi16 + B) EXACTLY (margin ~0.27; hi16 read as strided int16
  view of PSUM).  One DVE tensor_scalar does the affine + RNE cast; ACT
  Sqrt(v)+accum produces sum sqrt(v) partials; a 2-level tensor_tensor min
  tree gives v_neq = min_{c != g} v_c, then Sqrt+accum.

  Band +-3 drops D^2 > 18 configurations at frequency ~(3/4)^48 per pixel
  (expected <5 pixels across the whole batch, each off by <1.1 in d): error
  ~1e-6 on the loss.  Validated end-to-end in numpy at rel err 8.4e-5.

Per core: 2 images.  Output: [128, 16] f32 per-partition partial sums.
"""

import ml_dtypes
import numpy as np

import concourse.bacc as bacc
import concourse.bass as bass
import concourse.mybir as mybir
from concourse.mybir import AluOpType as Op
from concourse.tile import TileContext

P = 128
H = W = 512
YB = H // P          # 4 blocks per image dimension
C = 4                # classes
BPC = 2              # images per core
NCORES = 8
B_TOTAL = BPC * NCORES
RAD = 3              # conv band radius

BETA = 5.0
BW = 20.0            # per-pass exp bias (two passes -> e^40 at distance 0)
K_EXCL = 42.0        # own-class exclusion: subtract e^42 * m

A_COEF = -float(np.log(2.0)) / 640.0
B_COEF = (40.0 + 127.0 * float(np.log(2.0))) / 5.0 + 0.2277
GAMMA_HAT = float(np.sqrt(35.0))   # constant own-class cluster, verified

F32 = mybir.dt.float32
BF16 = mybir.dt.bfloat16
I32 = mybir.dt.int32
I16 = mybir.dt.int16
Act = mybir.ActivationFunctionType

# weight-matrix column layout in the packed wmats input
WCOMB_OFF, DIAG_OFF = 0, 134
WM_COLS = 134 + 128


def _band_val(d):
    if abs(d) > RAD:
        return 0.0
    return float(np.exp(BW - BETA * d * d))


def _host_wmats():
    """[128, 262] bf16: W_COMB [128,134] | DIAG [128,128] (-e^42).

    W_COMB[p, q] = band(q - 3 - p): matmul with stationary = data block j
    and moving = W_COMB writes output window [128j-3, 128j+131) — the
    3-col halo into each neighbor block plus the 128-col body, all from
    one quadrant-aligned full-height stationary.
    """
    wm = np.zeros((P, WM_COLS), np.float32)
    for p_ in range(P):
        for q in range(134):
            wm[p_, WCOMB_OFF + q] = _band_val(q - 3 - p_)
    for p_ in range(P):
        wm[p_, DIAG_OFF + p_] = -float(np.exp(K_EXCL))
    return wm.astype(ml_dtypes.bfloat16)


def _build_nc():
    nc = bacc.Bacc("TRN2", target_bir_lowering=False, debug=False)
    tgt_d = nc.dram_tensor("target", [BPC, H, W], I32, kind="ExternalInput")
    wm_d = nc.dram_tensor("wmats", [P, WM_COLS], BF16, kind="ExternalInput")
    osum_d = nc.dram_tensor("osum", [P, BPC * YB], F32, kind="ExternalOutput")

    with TileContext(nc) as tc:
        with (
            tc.tile_pool(name="const", bufs=1) as cpool,
            tc.tile_pool(name="tgt", bufs=4) as tgt_pool,
            tc.tile_pool(name="tgtb", bufs=8) as tb_pool,
            tc.tile_pool(name="mask", bufs=2 * YB) as m_pool,
            tc.tile_pool(name="st", bufs=2 * YB) as st_pool,
            tc.tile_pool(name="v16", bufs=3) as v_pool,
            tc.tile_pool(name="mins", bufs=3) as mn_pool,
            tc.tile_pool(name="scratch", bufs=2) as s_pool,
            tc.tile_pool(name="acc", bufs=1) as a_pool,
            tc.tile_pool(name="ps", bufs=2, space="PSUM") as ps_pool,
        ):
            wtile = cpool.tile([P, WM_COLS], BF16)
            nc.sync.dma_start(wtile, wm_d[:, :])
            wc = wtile[:, WCOMB_OFF:WCOMB_OFF + 134]
            w_diag = wtile[:, DIAG_OFF:DIAG_OFF + P]
            bias_b = cpool.tile([P, 1], F32)
            nc.vector.memset(bias_b, B_COEF)

            def banded_conv(ps_row, stat_of, fresh):
                """Banded conv into psum row `ps_row` ([128, W] f32);
                stat_of(j) gives the [128,128] stationary for block j.

                fresh=True: regions initialized by start=True pieces (7
                matmuls, 6-col overlap pieces accumulate).  fresh=False:
                the bank was already initialized (diag); 4 accumulating
                window matmuls, one per block."""
                for j in range(YB):
                    lo = j * P - 3
                    stat = stat_of(j)
                    last = j == YB - 1
                    if not fresh:
                        if j == 0:
                            nc.tensor.matmul(ps_row[:, 0:131], stat,
                                             wc[:, 3:134],
                                             start=False, stop=False,
                                             skip_group_check=True)
                        else:
                            hi_end = min(lo + 134, W)
                            nc.tensor.matmul(ps_row[:, lo:hi_end], stat,
                                             wc[:, 0:hi_end - lo],
                                             start=False, stop=last,
                                             skip_group_check=True)
                        continue
                    if j == 0:
                        nc.tensor.matmul(ps_row[:, 0:131], stat, wc[:, 3:134],
                                         start=True, stop=False,
                                         skip_group_check=True)
                        continue
                    hi_end = min(lo + 134, W)
                    nc.tensor.matmul(ps_row[:, lo:lo + 6], stat, wc[:, 0:6],
                                     start=False, stop=False,
                                     skip_group_check=True)
                    nc.tensor.matmul(ps_row[:, lo + 6:hi_end], stat,
                                     wc[:, 6:6 + (hi_end - lo - 6)],
                                     start=True, stop=last,
                                     skip_group_check=True)

            osum = a_pool.tile([P, BPC * YB], F32)

            def stage_load(b):
                """DMA targets + i32->bf16 casts (image 0 on the idle
                Scalar engine to shorten the pipeline head)."""
                tgtb = []
                for yb in range(YB):
                    tgt = tgt_pool.tile([P, W], I32)
                    nc.sync.dma_start(tgt, tgt_d[b, yb * P:(yb + 1) * P, :])
                    tb = tb_pool.tile([P, W], BF16)
                    if b == 0:
                        nc.scalar.activation(tb, tgt, Act.Copy)
                    else:
                        nc.gpsimd.tensor_copy(tb, tgt)
                    tgtb.append(tb)
                return tgtb

            def stage_masks(tgtb):
                mw = []
                for yb in range(YB):
                    mwt = m_pool.tile([P, C, W], BF16)
                    for c in range(C):
                        nc.vector.tensor_scalar(mwt[:, c], tgtb[yb], float(c),
                                                None, Op.is_equal)
                    mw.append(mwt)
                return mw

            def stage_pass1(mw):
                """y-conv fused with transpose; drain to bf16 (V/S split)."""
                st = []
                for xb in range(YB):
                    syt = ps_pool.tile([P, C, W], F32, tag="ps")
                    for c in range(C):
                        banded_conv(syt[:, c],
                                    lambda j: mw[j][:, c, xb * P:(xb + 1) * P],
                                    fresh=True)
                    stx = st_pool.tile([P, C, W], BF16)
                    nc.scalar.activation(stx, syt, Act.Copy)
                    st.append(stx)
                return st

            def stage_pass2(b, mw, st):
                """x-conv + diag exclusion; extract; min tree; fused sqrt."""
                for ys in range(YB):
                    xp = ps_pool.tile([P, C, W], F32, tag="ps")
                    for c in range(C):
                        nc.tensor.matmul(
                            xp[:, c, :], w_diag, mw[ys][:, c, :],
                            start=True, stop=False, skip_group_check=True,
                        )
                        banded_conv(xp[:, c],
                                    lambda xb: st[xb][:, c, ys * P:(ys + 1) * P],
                                    fresh=False)

                    # v = round(A*hi16 + B) from the PSUM top halves; planes
                    # 0..3 = classes, plane 4 = min tree for d_neq.
                    xpap = xp[:].bitcast(I16)
                    v16 = v_pool.tile([P, C + 1, W], I16)
                    hi = bass.AP(xpap.tensor, xpap.offset + 1,
                                 [xpap.ap[0], [2 * W, C], [2, W]])
                    if ys == YB - 1:
                        nc.scalar.activation(v16[:, 0:C], hi, Act.Identity,
                                             bias=bias_b[:], scale=A_COEF)
                    else:
                        nc.vector.tensor_scalar(v16[:, 0:C], hi, A_COEF,
                                                B_COEF, Op.mult, Op.add)

                    pr = mn_pool.tile([P, 2, W], I16)
                    nc.vector.tensor_tensor(pr[:], v16[:, 0:2], v16[:, 2:4],
                                            Op.min)
                    nc.vector.tensor_tensor(v16[:, C], pr[:, 0], pr[:, 1],
                                            Op.min)

                    col = b * YB + ys
                    dscr = s_pool.tile([P, C + 1, W], BF16)
                    nc.scalar.activation(dscr[:], v16[:], Act.Sqrt,
                                         accum_out=osum[:, col:col + 1])

            # two-image software pipeline: both pass-1s run back to back so
            # the Scalar drains never queue behind pass-2 sqrt work, and the
            # Vector mask work for image 1 hides under image 0's conv.
            tgtb0 = stage_load(0)
            tgtb1 = stage_load(1)
            mw0 = stage_masks(tgtb0)
            st0 = stage_pass1(mw0)
            mw1 = stage_masks(tgtb1)
            st1 = stage_pass1(mw1)
            stage_pass2(0, mw0, st0)
            stage_pass2(1, mw1, st1)

            nc.sync.dma_start(osum_d[:, :], osum)

    nc.compile()
    return nc


_NC = None
_WM = None


def _get_nc():
    global _NC
    if _NC is None:
        _NC = _build_nc()
    return _NC


def _get_wm():
    global _WM
    if _WM is None:
        _WM = _host_wmats()
    return _WM


def _exact_fallback(pred, target):
    """Exact numpy implementation of the reference (adversarial inputs only)."""
    THETA0, THETA, R = 3.0, 5.0, 5
    offs = [(dy, dx, float(np.hypot(dy, dx)))
            for dy in range(-R, R + 1) for dx in range(-R, R + 1)
            if np.hypot(dy, dx) <= THETA]

    def capped_edt(ts):
        B, Hh, Ww = ts.shape
        pad = np.zeros((B, Hh + 2 * R, Ww + 2 * R), bool)
        pad[:, R:-R, R:-R] = ts
        d = np.full((B, Hh, Ww), THETA, np.float32)
        for dy, dx, dist in offs:
            win = pad[:, R + dy:R + dy + Hh, R + dx:R + dx + Ww]
            d = np.minimum(d, np.where(win, np.float32(dist), np.float32(THETA)))
        return d

    def compute_sdf(mask):
        sdf_pos = capped_edt(mask == 1.0)
        sdf_neg = capped_edt(mask == 0.0)
        sdf = np.clip(sdf_pos - sdf_neg, -THETA, THETA) / THETA
        empty = mask.sum(axis=(1, 2)) == 0.0
        return np.where(empty[:, None, None], np.float32(THETA0), sdf).astype(np.float32)

    x = pred.astype(np.float32)
    x = x - x.max(axis=1, keepdims=True)
    ex = np.exp(x)
    p = ex / ex.sum(axis=1, keepdims=True)
    Cn = pred.shape[1]
    loss = np.float32(0.0)
    for c in range(Cn):
        ps = compute_sdf(p[:, c].astype(np.float32))
        ts = compute_sdf((target == c).astype(np.float32))
        loss += np.abs(ps - ts).mean(dtype=np.float32)
    return np.float32(loss / Cn)


def kernel(pred: np.ndarray, target: np.ndarray) -> np.ndarray:
    pred = np.asarray(pred)
    target = np.asarray(target)

    gap_ok = float(pred.max()) - float(pred.min()) < 15.0
    tgt_ok = bool(((target >= 0) & (target < C)).all())
    present = np.array([[(target[b] == c).any() for c in range(C)]
                        for b in range(B_TOTAL)])
    if not (gap_ok and tgt_ok and present.all()):
        return _exact_fallback(pred, target)

    from concourse.bass_utils import run_bass_kernel_spmd

    nc = _get_nc()
    wm = _get_wm()
    in_maps = [
        {"target": np.ascontiguousarray(target[i * BPC:(i + 1) * BPC]),
         "wmats": wm}
        for i in range(NCORES)
    ]
    try:
        res = run_bass_kernel_spmd(nc, in_maps, list(range(NCORES))).results
    except Exception:
        import time as _time
        _time.sleep(3.0)
        res = run_bass_kernel_spmd(nc, in_maps, list(range(NCORES))).results

    npx = H * W
    total = 0.0
    for core in range(NCORES):
        total += float(res[core]["osum"].astype(np.float64).sum())
    total -= B_TOTAL * npx * GAMMA_HAT
    loss = total / (5.0 * npx * B_TOTAL * C)
    return np.float32(loss)
